# Initial kernel scaffold
#
"""Trainium2 Bass kernel for nn_BubblePredictor (GRU recurrence + linear head).

Full problem: history [1024, 2048, 12] fp32, torch-GRUCell math (bias-free)
with H=512, per-step 2-unit head. Returns (logits [1024, 2048, 2], h [1024, 512]).

Sharding: data-parallel over batch across 8 NeuronCores (128 rows each);
GRU + head weights replicated; the time recurrence stays local per shard.

Per-core kernel design (per step t):
  - h kept batch-major [128, 512] fp32 (master) + bf16 copy + bf16 transposed
    copy hT (4 chunks of [128,128]) used as matmul stationary.
  - gates via PE into PSUM, gi (x @ w_ih.T) accumulated on top of gh (h @ w_hh.T)
    for the r/z banks by the PE itself; i_n kept in its own bank.
  - weight columns pre-arranged [r_h0 z_h0 | r_h1 z_h1 | n_h0 n_h1 | w_out(2)]
    so each PSUM bank is one N=512 accumulation group and consecutive matmuls
    alternate banks (fast PE mode). z columns are negated so one merged
    sigmoid yields both r and (1-z).
  - vector chain split into two 256-col halves for cross-engine overlap:
    sigmoid -> c = r*h_n -> d = c + i_n -> tanh -> e = n - h -> f = (1-z)*e
    -> h' = h + f (fp32) -> bf16 cast -> PE transposes (half 1 deferred into
    the next step's PE stream) -> hT.
  - head: h_t @ w_out.T accumulated 256 steps per PSUM bank, copied in bulk.
  - x is host-pre-transposed/bf16-cast to xt[replica, t, batch] so the
    per-step gi stationary is a plain SBUF slice.
"""

import sys

sys.path.insert(0, "/opt/trn_rl_repo")

from contextlib import ExitStack

import numpy as np
import ml_dtypes

import concourse.mybir as mybir
import concourse.tile as tile
from concourse import bacc
from concourse.bass_utils import run_bass_kernel_spmd
from concourse.masks import make_identity

F32 = mybir.dt.float32
BF16 = mybir.dt.bfloat16
AF = mybir.ActivationFunctionType

NCORES = 8
B = 128     # batch rows per core
H = 512
I = 12
K4 = 4
T = 2048
XT_CHUNK = 64


def _build(T=T, xt_chunk=XT_CHUNK):
    nc = bacc.Bacc(None, target_bir_lowering=False, debug=False)

    xt_d = nc.dram_tensor("xt", [96, T, B], BF16, kind="ExternalInput")
    wmov_d = nc.dram_tensor("wmov", [128, K4, 1538], BF16, kind="ExternalInput")
    wih_d = nc.dram_tensor("wih", [128, 1024], BF16, kind="ExternalInput")
    logits_d = nc.dram_tensor("logits", [B, T * 2], F32, kind="ExternalOutput")
    hout_d = nc.dram_tensor("hout", [B, H], F32, kind="ExternalOutput")

    assert T % xt_chunk == 0
    n_chunks = T // xt_chunk

    with tile.TileContext(nc) as tc:
        with ExitStack() as ctx:
            const = ctx.enter_context(tc.tile_pool(name="const", bufs=1))
            state = ctx.enter_context(tc.tile_pool(name="state", bufs=2))
            tmp = ctx.enter_context(tc.tile_pool(name="tmp", bufs=2))
            xtp = ctx.enter_context(tc.tile_pool(name="xtp", bufs=2))
            ps1 = ctx.enter_context(tc.tile_pool(name="ps1", bufs=1, space="PSUM"))
            ps2 = ctx.enter_context(tc.tile_pool(name="ps2", bufs=2, space="PSUM"))

            w_sb = const.tile([128, K4, 1538], BF16, tag="w_sb")
            wih_sb = const.tile([128, 1024], BF16, tag="wih_sb")
            ident = const.tile([128, 128], BF16, tag="ident")
            logit_sb = const.tile([B, T * 2], F32, tag="logit_sb")

            nc.sync.dma_start(out=w_sb[:], in_=wmov_d[:])
            nc.sync.dma_start(out=wih_sb[:], in_=wih_d[:])
            make_identity(nc, ident[:])

            h = state.tile([B, H], F32, tag="h")
            hb = state.tile([B, H], BF16, tag="hb")
            hT = state.tile([128, K4, 128], BF16, tag="hT")
            nc.vector.memset(h[:], 0.0)
            nc.vector.memset(hb[:], 0.0)
            nc.vector.memset(hT[:], 0.0)

            p_lg = None
            pend_transp = None
            for c_i in range(n_chunks):
                xt_sb = xtp.tile([96, xt_chunk * B], BF16, tag="xt")
                nc.sync.dma_start(
                    out=xt_sb[:],
                    in_=xt_d[:, c_i * xt_chunk : (c_i + 1) * xt_chunk, :],
                )
                for s in range(xt_chunk):
                    t = c_i * xt_chunk + s
                    xs = slice(s * B, (s + 1) * B)

                    p_rz = ps2.tile([B, 1024], F32, tag="p_rz")
                    p_nn = ps1.tile([B, 512], F32, tag="p_nn")
                    p_in = ps1.tile([B, 512], F32, tag="p_in")

                    rz = tmp.tile([B, 1024], BF16, tag="rz")
                    cd = tmp.tile([B, 512], F32, tag="cd")
                    n_t = tmp.tile([B, 512], BF16, tag="n_t")
                    e_t = tmp.tile([B, 512], BF16, tag="e_t")
                    f_t = tmp.tile([B, 512], BF16, tag="f_t")
                    h_new = state.tile([B, H], F32, tag="h")
                    hb_new = state.tile([B, H], BF16, tag="hb")
                    hT_new = state.tile([128, K4, 128], BF16, tag="hT")
                    p_ht = ps1.tile([128, K4 * 128], BF16, tag="p_ht")

                    dsts = (p_rz[:, 0:512], p_rz[:, 512:1024], p_nn[:])

                    # PE phase A: k=0,1 interleaved across the 3 gate banks
                    for k in (0, 1):
                        for m, dst in enumerate(dsts):
                            nc.tensor.matmul(
                                dst, hT[:, k, :],
                                w_sb[:, k, 512 * m : 512 * (m + 1)],
                                start=(k == 0), stop=False,
                            )

                    # deferred transposes of previous step's half 1
                    if pend_transp is not None:
                        hbp, hTp, p_htp = pend_transp
                        for k in (2, 3):
                            nc.tensor.transpose(
                                p_htp[:, 128 * k : 128 * (k + 1)],
                                hbp[:, 128 * k : 128 * (k + 1)],
                                ident[:],
                            )
                        nc.scalar.copy(hTp[:, 2:4, :], p_htp[:, 256:512])

                    # PE phase B: k=2,3 + gi + head
                    for k in (2, 3):
                        for m, dst in enumerate(dsts):
                            nc.tensor.matmul(
                                dst, hT[:, k, :],
                                w_sb[:, k, 512 * m : 512 * (m + 1)],
                                start=False, stop=(m == 2 and k == 3),
                            )
                    for m in (0, 1):
                        nc.tensor.matmul(
                            dsts[m], xt_sb[32 * m : 32 * m + I, xs],
                            wih_sb[32 * m : 32 * m + I, 0:512],
                            start=False, stop=True,
                        )
                    nc.tensor.matmul(
                        p_in[:], xt_sb[64 : 64 + I, xs], wih_sb[64 : 64 + I, 0:512],
                        start=True, stop=True,
                    )

                    if t > 0:
                        if (t - 1) % 256 == 0:
                            p_lg = ps1.tile([B, 512], F32, tag="p_lg")
                        o = ((t - 1) % 256) * 2
                        for k in range(K4):
                            nc.tensor.matmul(
                                p_lg[:, o : o + 2], hT[:, k, :],
                                w_sb[:, k, 1536:1538],
                                start=(k == 0), stop=(k == 3),
                            )
                        if (t - 1) % 256 == 255:
                            base = (t - 1) - 255
                            nc.scalar.copy(
                                logit_sb[:, 2 * base : 2 * (base + 256)], p_lg[:]
                            )

                    # vector chains, stage-major across the two 256-col halves
                    US = [slice(0, 256), slice(256, 512)]
                    RU = [rz[:, 0:256], rz[:, 512:768]]
                    ZU = [rz[:, 256:512], rz[:, 768:1024]]  # holds 1-z
                    for u in (0, 1):
                        nc.scalar.activation(
                            rz[:, 512 * u : 512 * (u + 1)],
                            p_rz[:, 512 * u : 512 * (u + 1)], AF.Sigmoid,
                        )
                    for u in (0, 1):
                        nc.vector.tensor_mul(cd[:, US[u]], RU[u], p_nn[:, US[u]])
                        nc.vector.tensor_add(cd[:, US[u]], cd[:, US[u]], p_in[:, US[u]])
                    for u in (0, 1):
                        nc.scalar.activation(n_t[:, US[u]], cd[:, US[u]], AF.Tanh)
                    for u in (0, 1):
                        nc.vector.tensor_sub(e_t[:, US[u]], n_t[:, US[u]], hb[:, US[u]])
                        nc.vector.tensor_mul(f_t[:, US[u]], ZU[u], e_t[:, US[u]])
                        nc.vector.tensor_add(h_new[:, US[u]], h[:, US[u]], f_t[:, US[u]])
                        nc.vector.tensor_copy(hb_new[:, US[u]], h_new[:, US[u]])

                    # transposes: half 0 now, half 1 deferred into next step
                    for k in (0, 1):
                        nc.tensor.transpose(
                            p_ht[:, 128 * k : 128 * (k + 1)],
                            hb_new[:, 128 * k : 128 * (k + 1)],
                            ident[:],
                        )
                    nc.scalar.copy(hT_new[:, 0:2, :], p_ht[:, 0:256])
                    pend_transp = (hb_new, hT_new, p_ht)

                    h, hb, hT = h_new, hb_new, hT_new

            if pend_transp is not None:
                hbp, hTp, p_htp = pend_transp
                for k in (2, 3):
                    nc.tensor.transpose(
                        p_htp[:, 128 * k : 128 * (k + 1)],
                        hbp[:, 128 * k : 128 * (k + 1)],
                        ident[:],
                    )
                nc.scalar.copy(hTp[:, 2:4, :], p_htp[:, 256:512])

            # final head (logit for step T-1 uses final h)
            o = ((T - 1) % 256) * 2
            if (T - 1) % 256 == 0:
                p_lg = ps1.tile([B, 512], F32, tag="p_lg")
            for k in range(K4):
                nc.tensor.matmul(
                    p_lg[:, o : o + 2], hT[:, k, :], w_sb[:, k, 1536:1538],
                    start=(k == 0), stop=(k == 3),
                )
            base = (T - 1) - (T - 1) % 256
            nc.scalar.copy(
                logit_sb[:, 2 * base : 2 * T], p_lg[:, : ((T - 1) % 256 + 1) * 2]
            )

            nc.sync.dma_start(out=logits_d[:], in_=logit_sb[:])
            nc.sync.dma_start(out=hout_d[:], in_=h[:])

    nc.compile()
    return nc


def _reorder_cols(a):
    """[..., 1536] gate cols [r z n] -> [r0 z0 r1 z1 n0 n1], z negated
    (merged sigmoid then yields 1-z for the z slots)."""
    r, z, n = a[..., 0:512], -a[..., 512:1024], a[..., 1024:1536]
    return np.concatenate(
        [r[..., 0:256], z[..., 0:256], r[..., 256:512], z[..., 256:512],
         n[..., 0:256], n[..., 256:512]], axis=-1)


def _prep_weights(w_ih, w_hh, w_out):
    wt = np.concatenate([w_hh, w_out], axis=0).T  # [512, 1538]
    wt = np.concatenate([_reorder_cols(wt[:, 0:1536]), wt[:, 1536:1538]], axis=1)
    wmov = np.ascontiguousarray(wt.reshape(K4, 128, 1538)).astype(ml_dtypes.bfloat16)
    wmov = np.ascontiguousarray(wmov.transpose(1, 0, 2))  # [128, k, 1538]

    wihT = _reorder_cols(w_ih.T)  # [12, 1536]
    wih = np.zeros((128, 1024), dtype=ml_dtypes.bfloat16)
    wih[0:I, 0:512] = wihT[:, 0:512]          # [r0|z0] -> bank rz0
    wih[32 : 32 + I, 0:512] = wihT[:, 512:1024]   # [r1|z1] -> bank rz1
    wih[64 : 64 + I, 0:512] = wihT[:, 1024:1536]  # [n0|n1] -> p_in
    return wmov, wih


def _prep_xt(history_c):
    """history_c [128, T, 12] fp32 -> xt [96, T, 128] bf16 (3 replicas)."""
    Tc = history_c.shape[1]
    xt = np.zeros((96, Tc, B), dtype=ml_dtypes.bfloat16)
    xT = np.ascontiguousarray(history_c.transpose(2, 1, 0))
    for m in range(3):
        xt[32 * m : 32 * m + I] = xT
    return xt


_NC_CACHE = {}


def kernel(history, w_ih, w_hh, w_out, b_out):
    history = np.asarray(history, dtype=np.float32)
    w_ih = np.asarray(w_ih, dtype=np.float32)
    w_hh = np.asarray(w_hh, dtype=np.float32)
    w_out = np.asarray(w_out, dtype=np.float32)
    b_out = np.asarray(b_out, dtype=np.float32)

    Bf, Tf, If = history.shape
    assert (Bf, Tf, If) == (NCORES * B, T, I), (Bf, Tf, If)

    if "nc" not in _NC_CACHE:
        _NC_CACHE["nc"] = _build()
    nc = _NC_CACHE["nc"]

    wmov, wih = _prep_weights(w_ih, w_hh, w_out)
    in_maps = []
    for c in range(NCORES):
        in_maps.append({
            "xt": _prep_xt(history[c * B : (c + 1) * B]),
            "wmov": wmov,
            "wih": wih,
        })

    res = run_bass_kernel_spmd(nc, in_maps, core_ids=list(range(NCORES)))

    logits = np.concatenate(
        [res.results[c]["logits"].reshape(B, T, 2) for c in range(NCORES)], axis=0
    )
    h_final = np.concatenate(
        [res.results[c]["hout"] for c in range(NCORES)], axis=0
    )
    logits = logits + b_out[None, None, :]
    return logits.astype(np.float32), h_final.astype(np.float32)


# revision 1
# speedup vs baseline: 1.1584x; 1.1584x over previous
"""Trainium2 Bass kernel for nn_BubblePredictor (GRU recurrence + linear head).

Full problem: history [1024, 2048, 12] fp32, torch-GRUCell math (bias-free)
with H=512, per-step 2-unit head. Returns (logits [1024, 2048, 2], h [1024, 512]).

Sharding: data-parallel over batch across 8 NeuronCores (128 rows each);
GRU + head weights replicated; the time recurrence stays local per shard.

Per-core kernel design (per step t):
  - h kept batch-major [128, 512] fp32 (master) + bf16 copy + bf16 transposed
    copy hT (4 chunks of [128,128]) used as matmul stationary.
  - gates via PE into PSUM, gi (x @ w_ih.T) accumulated on top of gh (h @ w_hh.T)
    for the r/z banks by the PE itself; i_n kept in its own bank.
  - weight columns pre-arranged [r_h0 z_h0 | r_h1 z_h1 | n_h0 n_h1 | w_out(2)]
    so each PSUM bank is one N=512 accumulation group and consecutive matmuls
    alternate banks (fast PE mode). z columns are negated so one merged
    sigmoid yields both r and (1-z).
  - vector chain split into two 256-col halves for cross-engine overlap:
    sigmoid -> c = r*h_n -> d = c + i_n -> tanh -> e = n - h -> f = (1-z)*e
    -> h' = h + f (fp32) -> bf16 cast -> PE transposes (half 1 deferred into
    the next step's PE stream) -> hT.
  - head: h_t @ w_out.T accumulated 256 steps per PSUM bank, copied in bulk.
  - x is host-pre-transposed/bf16-cast to xt[replica, t, batch] so the
    per-step gi stationary is a plain SBUF slice.
"""

import sys

sys.path.insert(0, "/opt/trn_rl_repo")

from contextlib import ExitStack

import numpy as np
import ml_dtypes

import concourse.mybir as mybir
import concourse.tile as tile
from concourse import bacc
from concourse.bass_utils import run_bass_kernel_spmd
from concourse.masks import make_identity

F32 = mybir.dt.float32
BF16 = mybir.dt.bfloat16
AF = mybir.ActivationFunctionType

NCORES = 8
B = 128     # batch rows per core
H = 512
I = 12
K4 = 4
T = 2048
XT_CHUNK = 64


def _build(T=T, xt_chunk=XT_CHUNK):
    nc = bacc.Bacc(None, target_bir_lowering=False, debug=False)

    xt_d = nc.dram_tensor("xt", [96, T, B], BF16, kind="ExternalInput")
    wmov_d = nc.dram_tensor("wmov", [128, K4, 1538], BF16, kind="ExternalInput")
    wih_d = nc.dram_tensor("wih", [128, 1024], BF16, kind="ExternalInput")
    logits_d = nc.dram_tensor("logits", [B, T * 2], F32, kind="ExternalOutput")
    hout_d = nc.dram_tensor("hout", [B, H], F32, kind="ExternalOutput")

    assert T % xt_chunk == 0
    n_chunks = T // xt_chunk

    with tile.TileContext(nc) as tc:
        with ExitStack() as ctx:
            const = ctx.enter_context(tc.tile_pool(name="const", bufs=1))
            state = ctx.enter_context(tc.tile_pool(name="state", bufs=2))
            tmp = ctx.enter_context(tc.tile_pool(name="tmp", bufs=2))
            xtp = ctx.enter_context(tc.tile_pool(name="xtp", bufs=2))
            ps1 = ctx.enter_context(tc.tile_pool(name="ps1", bufs=1, space="PSUM"))
            ps2 = ctx.enter_context(tc.tile_pool(name="ps2", bufs=2, space="PSUM"))

            w_sb = const.tile([128, K4, 1538], BF16, tag="w_sb")
            wih_sb = const.tile([128, 1024], BF16, tag="wih_sb")
            ident = const.tile([128, 128], BF16, tag="ident")
            logit_sb = const.tile([B, T * 2], F32, tag="logit_sb")

            nc.sync.dma_start(out=w_sb[:], in_=wmov_d[:])
            nc.sync.dma_start(out=wih_sb[:], in_=wih_d[:])
            make_identity(nc, ident[:])

            h = state.tile([B, H], F32, tag="h")
            hb = state.tile([B, H], BF16, tag="hb")
            hT = state.tile([128, K4, 128], BF16, tag="hT")
            nc.vector.memset(h[:], 0.0)
            nc.vector.memset(hb[:], 0.0)
            nc.vector.memset(hT[:], 0.0)

            p_lg = None
            pend_transp = None
            for c_i in range(n_chunks):
                xt_sb = xtp.tile([96, xt_chunk * B], BF16, tag="xt")
                nc.sync.dma_start(
                    out=xt_sb[:],
                    in_=xt_d[:, c_i * xt_chunk : (c_i + 1) * xt_chunk, :],
                )
                for s in range(xt_chunk):
                    t = c_i * xt_chunk + s
                    xs = slice(s * B, (s + 1) * B)

                    p_rz = ps2.tile([B, 1024], F32, tag="p_rz")
                    p_nn = ps1.tile([B, 512], F32, tag="p_nn")
                    p_in = ps1.tile([B, 512], F32, tag="p_in")

                    rz = tmp.tile([B, 1024], BF16, tag="rz")
                    cd = tmp.tile([B, 512], F32, tag="cd")
                    n_t = tmp.tile([B, 512], BF16, tag="n_t")
                    e_t = tmp.tile([B, 512], BF16, tag="e_t")
                    f_t = tmp.tile([B, 512], BF16, tag="f_t")
                    h_new = state.tile([B, H], F32, tag="h")
                    hb_new = state.tile([B, H], BF16, tag="hb")
                    hT_new = state.tile([128, K4, 128], BF16, tag="hT")
                    p_ht = ps1.tile([128, K4 * 128], BF16, tag="p_ht")

                    dsts = (p_rz[:, 0:512], p_rz[:, 512:1024], p_nn[:])

                    # PE phase A: k=0,1 interleaved across the 3 gate banks
                    for k in (0, 1):
                        for m, dst in enumerate(dsts):
                            nc.tensor.matmul(
                                dst, hT[:, k, :],
                                w_sb[:, k, 512 * m : 512 * (m + 1)],
                                start=(k == 0), stop=False,
                            )

                    # deferred transposes of previous step's half 1
                    if pend_transp is not None:
                        hbp, hTp, p_htp = pend_transp
                        for k in (2, 3):
                            nc.tensor.transpose(
                                p_htp[:, 128 * k : 128 * (k + 1)],
                                hbp[:, 128 * k : 128 * (k + 1)],
                                ident[:],
                            )
                        nc.scalar.copy(hTp[:, 2:4, :], p_htp[:, 256:512])

                    # PE phase B: k=2,3 + gi + head
                    for k in (2, 3):
                        for m, dst in enumerate(dsts):
                            nc.tensor.matmul(
                                dst, hT[:, k, :],
                                w_sb[:, k, 512 * m : 512 * (m + 1)],
                                start=False, stop=(m == 2 and k == 3),
                            )
                    for m in (0, 1):
                        nc.tensor.matmul(
                            dsts[m], xt_sb[32 * m : 32 * m + I, xs],
                            wih_sb[32 * m : 32 * m + I, 0:512],
                            start=False, stop=True,
                        )
                    nc.tensor.matmul(
                        p_in[:], xt_sb[64 : 64 + I, xs], wih_sb[64 : 64 + I, 0:512],
                        start=True, stop=True,
                    )

                    if t > 0:
                        if (t - 1) % 256 == 0:
                            p_lg = ps1.tile([B, 512], F32, tag="p_lg")
                        o = ((t - 1) % 256) * 2
                        for k in range(K4):
                            nc.tensor.matmul(
                                p_lg[:, o : o + 2], hT[:, k, :],
                                w_sb[:, k, 1536:1538],
                                start=(k == 0), stop=(k == 3),
                            )
                        if (t - 1) % 256 == 255:
                            base = (t - 1) - 255
                            nc.scalar.copy(
                                logit_sb[:, 2 * base : 2 * (base + 256)], p_lg[:]
                            )

                    # vector chains, stage-major across the two 256-col halves
                    US = [slice(0, 256), slice(256, 512)]
                    RU = [rz[:, 0:256], rz[:, 512:768]]
                    ZU = [rz[:, 256:512], rz[:, 768:1024]]  # holds 1-z
                    for u in (0, 1):
                        nc.scalar.activation(
                            rz[:, 512 * u : 512 * (u + 1)],
                            p_rz[:, 512 * u : 512 * (u + 1)], AF.Sigmoid,
                        )
                    for u in (0, 1):
                        nc.vector.tensor_mul(cd[:, US[u]], RU[u], p_nn[:, US[u]])
                        nc.vector.tensor_add(cd[:, US[u]], cd[:, US[u]], p_in[:, US[u]])
                    for u in (0, 1):
                        nc.scalar.activation(n_t[:, US[u]], cd[:, US[u]], AF.Tanh)
                    for u in (0, 1):
                        nc.vector.tensor_sub(e_t[:, US[u]], n_t[:, US[u]], hb[:, US[u]])
                        nc.vector.tensor_mul(f_t[:, US[u]], ZU[u], e_t[:, US[u]])
                        nc.vector.tensor_add(h_new[:, US[u]], h[:, US[u]], f_t[:, US[u]])
                        nc.vector.tensor_copy(hb_new[:, US[u]], h_new[:, US[u]])

                    # transposes: half 0 now, half 1 deferred into next step
                    for k in (0, 1):
                        nc.tensor.transpose(
                            p_ht[:, 128 * k : 128 * (k + 1)],
                            hb_new[:, 128 * k : 128 * (k + 1)],
                            ident[:],
                        )
                    nc.scalar.copy(hT_new[:, 0:2, :], p_ht[:, 0:256])
                    pend_transp = (hb_new, hT_new, p_ht)

                    h, hb, hT = h_new, hb_new, hT_new

            if pend_transp is not None:
                hbp, hTp, p_htp = pend_transp
                for k in (2, 3):
                    nc.tensor.transpose(
                        p_htp[:, 128 * k : 128 * (k + 1)],
                        hbp[:, 128 * k : 128 * (k + 1)],
                        ident[:],
                    )
                nc.scalar.copy(hTp[:, 2:4, :], p_htp[:, 256:512])

            # final head (logit for step T-1 uses final h)
            o = ((T - 1) % 256) * 2
            if (T - 1) % 256 == 0:
                p_lg = ps1.tile([B, 512], F32, tag="p_lg")
            for k in range(K4):
                nc.tensor.matmul(
                    p_lg[:, o : o + 2], hT[:, k, :], w_sb[:, k, 1536:1538],
                    start=(k == 0), stop=(k == 3),
                )
            base = (T - 1) - (T - 1) % 256
            nc.scalar.copy(
                logit_sb[:, 2 * base : 2 * T], p_lg[:, : ((T - 1) % 256 + 1) * 2]
            )

            nc.sync.dma_start(out=logits_d[:], in_=logit_sb[:])
            nc.sync.dma_start(out=hout_d[:], in_=h[:])

    nc.compile()
    return nc


def _reorder_cols(a):
    """[..., 1536] gate cols [r z n] -> [r0 z0 r1 z1 n0 n1], z negated
    (merged sigmoid then yields 1-z for the z slots)."""
    r, z, n = a[..., 0:512], -a[..., 512:1024], a[..., 1024:1536]
    return np.concatenate(
        [r[..., 0:256], z[..., 0:256], r[..., 256:512], z[..., 256:512],
         n[..., 0:256], n[..., 256:512]], axis=-1)


def _prep_weights(w_ih, w_hh, w_out):
    wt = np.concatenate([w_hh, w_out], axis=0).T  # [512, 1538]
    wt = np.concatenate([_reorder_cols(wt[:, 0:1536]), wt[:, 1536:1538]], axis=1)
    wmov = np.ascontiguousarray(wt.reshape(K4, 128, 1538)).astype(ml_dtypes.bfloat16)
    wmov = np.ascontiguousarray(wmov.transpose(1, 0, 2))  # [128, k, 1538]

    wihT = _reorder_cols(w_ih.T)  # [12, 1536]
    wih = np.zeros((128, 1024), dtype=ml_dtypes.bfloat16)
    wih[0:I, 0:512] = wihT[:, 0:512]          # [r0|z0] -> bank rz0
    wih[32 : 32 + I, 0:512] = wihT[:, 512:1024]   # [r1|z1] -> bank rz1
    wih[64 : 64 + I, 0:512] = wihT[:, 1024:1536]  # [n0|n1] -> p_in
    return wmov, wih


def _prep_xt(history_c):
    """history_c [128, T, 12] fp32 -> xt [96, T, 128] bf16 (3 replicas)."""
    Tc = history_c.shape[1]
    xt = np.zeros((96, Tc, B), dtype=ml_dtypes.bfloat16)
    xT = np.ascontiguousarray(history_c.transpose(2, 1, 0))
    for m in range(3):
        xt[32 * m : 32 * m + I] = xT
    return xt


_NC_CACHE = {}


def kernel(history, w_ih, w_hh, w_out, b_out):
    history = np.asarray(history, dtype=np.float32)
    w_ih = np.asarray(w_ih, dtype=np.float32)
    w_hh = np.asarray(w_hh, dtype=np.float32)
    w_out = np.asarray(w_out, dtype=np.float32)
    b_out = np.asarray(b_out, dtype=np.float32)

    Bf, Tf, If = history.shape
    assert (Bf, Tf, If) == (NCORES * B, T, I), (Bf, Tf, If)

    if "nc" not in _NC_CACHE:
        _NC_CACHE["nc"] = _build()
    nc = _NC_CACHE["nc"]

    wmov, wih = _prep_weights(w_ih, w_hh, w_out)
    in_maps = []
    for c in range(NCORES):
        in_maps.append({
            "xt": _prep_xt(history[c * B : (c + 1) * B]),
            "wmov": wmov,
            "wih": wih,
        })

    res = run_bass_kernel_spmd(nc, in_maps, core_ids=list(range(NCORES)))

    logits = np.concatenate(
        [res.results[c]["logits"].reshape(B, T, 2) for c in range(NCORES)], axis=0
    )
    h_final = np.concatenate(
        [res.results[c]["hout"] for c in range(NCORES)], axis=0
    )
    logits = logits + b_out[None, None, :]
    return logits.astype(np.float32), h_final.astype(np.float32)
